# revision 3
# baseline (speedup 1.0000x reference)
"""GAT (2-layer) + mean-pool + linear head on 8 Trainium2 NeuronCores.

Single SPMD launch, data-parallel over graphs (contiguous node ranges per
core, batch is sorted):
  - node phase: each core computes h1aug = x @ [W1 | W1 a_s | W1 a_d] for its
    own nodes and writes bf16 row-tables to DRAM:
       S1own[r] = [1, h1(64), as1, 0...]   (256B rows, gathered by src)
       D1[r]    = [ad1, r%128, 0...]       (gathered by dst, local ids)
  - AllGather S1own -> S1full (every core can gather any source row)
  - layer-1 edge phase: per dst tile, dma_gather src rows (edges pre-split
    into lo/hi index groups so int16 indices fit) + dst rows; compute
    el = exp(leaky_relu(as+ad)); one-hot scatter matmuls accumulate
    [denom | sum el*h] per dst tile in PSUM; epilogue normalizes, applies
    bias+relu, runs the layer-2 node matmul and writes S2own/D2 rows.
  - AllGather S2own -> S2full; layer-2 edge phase identical, epilogue feeds
    a pooling one-hot matmul (graph-mean) accumulated across tiles, then
    the linear head. Output: logits [10, 128] per core.

All edge-structure indices are computed on host (untimed) and uploaded as
int16 streams (~0.5MB/core); feature data never round-trips via host.
"""

import sys

sys.path.insert(0, "/opt/trn_rl_repo")

import numpy as np
import ml_dtypes

import concourse.bacc as bacc
import concourse.mybir as mybir
import concourse.tile as tile
from concourse import bass_utils
from concourse.masks import make_identity

F32 = mybir.dt.float32
BF16 = mybir.dt.bfloat16
I16 = mybir.dt.int16

N = 50000
E = 800000
F_IN, F_HID, F_OUT, N_CLS = 128, 64, 64, 10
N_GRAPHS = 512
NEG_SLOPE = 0.2
EPS = 1e-16
N_CORES = 8
P = 128
G_SLOTS = 128
REC = 128            # bf16 row width (256B) of all tables
HALF = 32768         # int16 index reach

_cache = {}
LAST_LAUNCH_WALLS = []


def _run(nc, in_maps, cores):
    import time
    t0 = time.time()
    res = bass_utils.run_bass_kernel_spmd(nc, in_maps, core_ids=cores)
    LAST_LAUNCH_WALLS.append(time.time() - t0)
    return res


def build(R_own, nb_lo, nb_hi, x_f32=True):
    """One SPMD program for all 8 cores.

    R_own: rows per core block in the tables (own tiles * 128 + 128 pad).
    nb_lo/nb_hi: per dst tile, number of 128-slot columns for edges whose
    source row id is < HALF (lo) / >= HALF (hi). Uniform across cores.
    """
    n_tiles = len(nb_lo)
    R_full = N_CORES * R_own
    T1_BASE = R_full - HALF          # hi-gather view base
    nb = [int(nb_lo[t] + nb_hi[t]) for t in range(n_tiles)]
    active = [t for t in range(n_tiles) if nb[t] > 0]
    cols_pre = np.concatenate([[0], np.cumsum(nb)]).astype(int)
    TB = int(cols_pre[-1])
    XD = BF16

    nc = bacc.Bacc("TRN2", target_bir_lowering=False, debug=False,
                   num_devices=N_CORES)
    # consolidated inputs: one array per dtype (each extra input array costs
    # ~100ms of axon transfer overhead per launch)
    NXT = R_own - P
    C1 = NXT + (F_HID + 2) + (F_OUT + 2) + N_CLS + 1 + F_HID + F_OUT         + n_tiles + 1
    blobf_in = nc.dram_tensor("blobf", [P, C1], BF16, kind="ExternalInput").ap()
    blobb_in = nc.dram_tensor("blobb", [P, 4 * REC], BF16,
                              kind="ExternalInput").ap()
    blobi_in = nc.dram_tensor("blobi", [16, 2 * TB * 8], I16,
                              kind="ExternalInput").ap()
    o = NXT
    xT = blobf_in[:, 0:NXT]
    w1 = blobf_in[:, o:o + F_HID + 2]; o += F_HID + 2
    w2 = blobf_in[0:F_HID, o:o + F_OUT + 2]; o += F_OUT + 2
    wl = blobf_in[0:F_OUT, o:o + N_CLS]; o += N_CLS
    bl = blobf_in[0:N_CLS, o:o + 1]; o += 1
    b1r = blobf_in[:, o:o + F_HID]; o += F_HID
    b2r = blobf_in[:, o:o + F_OUT]; o += F_OUT
    gid_in = blobf_in[:, o:o + n_tiles]; o += n_tiles
    rc_in = blobf_in[0:G_SLOTS, o:o + 1]; o += 1
    assert o == C1
    iota_in = blobb_in[:, 0:REC]
    stpl_in = blobb_in[:, REC:2 * REC]
    dtpl_in = blobb_in[:, 2 * REC:3 * REC]
    dsen_in = blobb_in[:, 3 * REC:4 * REC]
    idxS_in = blobi_in[:, 0:TB * 8]
    idxD_in = blobi_in[:, TB * 8:2 * TB * 8]
    out = nc.dram_tensor("logits", [N_CLS, G_SLOTS], F32,
                         kind="ExternalOutput").ap()

    with tile.TileContext(nc) as tc:
        with (
            tc.tile_pool(name="big", bufs=1) as big,
            tc.tile_pool(name="dram", bufs=1, space="DRAM") as dram,
            tc.tile_pool(name="sb", bufs=3) as sb,
            tc.tile_pool(name="gs", bufs=3) as gsp,
            tc.tile_pool(name="oh", bufs=6) as ohp,
            tc.tile_pool(name="acc", bufs=2, space="PSUM") as accp,
            tc.tile_pool(name="ptp", bufs=1, space="PSUM") as ptp,
            tc.tile_pool(name="pn2", bufs=1, space="PSUM") as pn2,
            tc.tile_pool(name="ppl", bufs=1, space="PSUM") as ppl,
        ):
            # ---------------- persistent small tensors
            iota_t = big.tile([P, P], BF16)
            nc.sync.dma_start(iota_t[:], iota_in)
            stpl_t = big.tile([P, REC], BF16)
            nc.sync.dma_start(stpl_t[:], stpl_in)
            dtpl_t = big.tile([P, REC], BF16)
            nc.sync.dma_start(dtpl_t[:], dtpl_in)
            dsen_t = big.tile([P, REC], BF16)
            nc.sync.dma_start(dsen_t[:], dsen_in)
            w1_t = big.tile([P, F_HID + 2], XD)
            nc.sync.dma_start(w1_t[:], w1)
            w2_t = big.tile([F_HID, F_OUT + 2], BF16)
            nc.sync.dma_start(w2_t[:], w2)
            wl_t = big.tile([F_OUT, N_CLS], BF16)
            nc.sync.dma_start(wl_t[:], wl)
            blb = big.tile([N_CLS, 1], BF16)
            nc.sync.dma_start(blb[:], bl)
            bl_t = big.tile([N_CLS, 1], F32)
            nc.vector.tensor_copy(bl_t[:], blb[:])
            b1_t = big.tile([P, F_HID], BF16)
            nc.sync.dma_start(b1_t[:], b1r)
            b2_t = big.tile([P, F_OUT], BF16)
            nc.sync.dma_start(b2_t[:], b2r)
            gidb = big.tile([P, n_tiles], BF16)
            nc.sync.dma_start(gidb[:], gid_in)
            gid_t = big.tile([P, n_tiles], F32)
            nc.vector.tensor_copy(gid_t[:], gidb[:])
            rcb = big.tile([G_SLOTS, 1], BF16)
            nc.sync.dma_start(rcb[:], rc_in)
            rc_t = big.tile([G_SLOTS, 1], F32)
            nc.vector.tensor_copy(rc_t[:], rcb[:])
            ident = big.tile([P, P], F32)
            make_identity(nc, ident[:])
            # idx streams live in SBUF; wrap layout [16, TB*8] replicated x8
            idxS_t = big.tile([P, TB * 8], I16)
            idxD_t = big.tile([P, TB * 8], I16)
            for g in range(8):
                nc.sync.dma_start(idxS_t[16 * g:16 * (g + 1), :], idxS_in)
                nc.sync.dma_start(idxD_t[16 * g:16 * (g + 1), :], idxD_in)

            # ---------------- DRAM tables
            S1own = dram.tile([R_own, REC], BF16)
            S2own = dram.tile([R_own, REC], BF16)
            D1 = dram.tile([R_own, REC], BF16)
            D2 = dram.tile([R_own, REC], BF16)
            S1full = dram.tile([R_full, REC], BF16)
            S2full = dram.tile([R_full, REC], BF16)

            # ---------------- node phase: own h1aug, S1own/D1 rows
            n_own_tiles = (R_own - P) // P
            for t in range(n_own_tiles):
                xt = sb.tile([P, P], XD, tag="xt")
                nc.sync.dma_start(xt[:], xT[:, t * P:(t + 1) * P])
                pn = pn2.tile([P, F_HID + 2], F32, tag="pn")
                nc.tensor.matmul(pn[:], lhsT=xt[:], rhs=w1_t[:],
                                 start=True, stop=True)
                rs = sb.tile([P, REC], BF16, tag="rs")
                nc.vector.tensor_copy(rs[:], stpl_t[:])
                nc.vector.tensor_copy(rs[:, 1:F_HID + 2], pn[:, :F_HID + 1])
                nc.sync.dma_start(S1own[t * P:(t + 1) * P, :], rs[:])
                rd = sb.tile([P, REC], BF16, tag="rd")
                nc.vector.tensor_copy(rd[:], dtpl_t[:])
                nc.vector.tensor_copy(rd[:, 0:1], pn[:, F_HID + 1:F_HID + 2])
                nc.sync.dma_start(D1[t * P:(t + 1) * P, :], rd[:])
            # pad tile: D sentinel rows (ad=0, dlmod=200); S pad rows benign
            nc.sync.dma_start(D1[n_own_tiles * P:(n_own_tiles + 1) * P, :],
                              dsen_t[:])
            nc.sync.dma_start(D2[n_own_tiles * P:(n_own_tiles + 1) * P, :],
                              dsen_t[:])
            nc.sync.dma_start(S1own[n_own_tiles * P:(n_own_tiles + 1) * P, :],
                              stpl_t[:])
            nc.sync.dma_start(S2own[n_own_tiles * P:(n_own_tiles + 1) * P, :],
                              stpl_t[:])

            nc.gpsimd.collective_compute(
                "AllGather", mybir.AluOpType.bypass,
                replica_groups=[list(range(N_CORES))],
                ins=[S1own[:]], outs=[S1full[:]])

            # ---------------- edge phases
            def edge_phase(Sfull, Dloc, layer):
                if layer == 2:
                    pool_ps = ppl.tile([G_SLOTS, F_OUT], F32)
                for ai, t in enumerate(active):
                    nbt = nb[t]
                    c0 = int(cols_pre[t])
                    gst = gsp.tile([P, nbt, REC], BF16, tag="gs")
                    gdt = gsp.tile([P, nbt, REC], BF16, tag="gd")
                    nlo, nhi = int(nb_lo[t]), int(nb_hi[t])
                    if nlo:
                        nc.gpsimd.dma_gather(
                            gst[:, 0:nlo, :], Sfull[0:HALF, :],
                            idxS_t[:, c0 * 8:(c0 + nlo) * 8],
                            num_idxs=nlo * P, num_idxs_reg=nlo * P,
                            elem_size=REC, single_packet=False)
                    if nhi:
                        nc.gpsimd.dma_gather(
                            gst[:, nlo:nbt, :], Sfull[T1_BASE:R_full, :],
                            idxS_t[:, (c0 + nlo) * 8:(c0 + nbt) * 8],
                            num_idxs=nhi * P, num_idxs_reg=nhi * P,
                            elem_size=REC, single_packet=False)
                    nc.gpsimd.dma_gather(
                        gdt[:, 0:nbt, :], Dloc[:, :],
                        idxD_t[:, c0 * 8:(c0 + nbt) * 8],
                        num_idxs=nbt * P, num_idxs_reg=nbt * P,
                        elem_size=REC, single_packet=False)
                    # compact per-slot scalars
                    z_t = sb.tile([P, nbt], F32, tag="z")
                    nc.vector.tensor_tensor(
                        out=z_t[:], in0=gst[:, :, F_HID + 1:F_HID + 2],
                        in1=gdt[:, :, 0:1], op=mybir.AluOpType.add)
                    dl_t = sb.tile([P, nbt], F32, tag="dl")
                    nc.vector.tensor_copy(dl_t[:], gdt[:, :, 1:2])
                    tmp = sb.tile([P, nbt], F32, tag="tmp")
                    nc.vector.tensor_scalar_mul(tmp[:], z_t[:], NEG_SLOPE)
                    nc.vector.tensor_tensor(out=tmp[:], in0=tmp[:], in1=z_t[:],
                                            op=mybir.AluOpType.max)
                    el_t = sb.tile([P, nbt], F32, tag="el")
                    nc.scalar.activation(el_t[:], tmp[:],
                                         mybir.ActivationFunctionType.Exp)
                    acc = accp.tile([P, F_HID + 1], F32, tag="acc")
                    for c in range(nbt):
                        oh = ohp.tile([P, P], BF16, tag="oh")
                        nc.vector.tensor_scalar(
                            oh[:], iota_t[:], dl_t[:, c:c + 1],
                            el_t[:, c:c + 1],
                            mybir.AluOpType.is_equal, mybir.AluOpType.mult)
                        nc.tensor.matmul(acc[:], lhsT=oh[:],
                                         rhs=gst[:, c:c + 1, 0:F_HID + 1],
                                         start=(c == 0), stop=(c == nbt - 1))
                    # epilogue
                    den = sb.tile([P, 1], F32, tag="den")
                    nc.vector.tensor_scalar_add(den[:], acc[:, 0:1], EPS)
                    rec = sb.tile([P, 1], F32, tag="rec")
                    nc.vector.reciprocal(rec[:], den[:])
                    o1 = sb.tile([P, F_HID], F32, tag="o1")
                    nc.vector.tensor_scalar_mul(o1[:], acc[:, 1:], rec[:, :1])
                    if layer == 1:
                        nc.vector.tensor_tensor(out=o1[:], in0=o1[:],
                                                in1=b1_t[:],
                                                op=mybir.AluOpType.add)
                        nc.scalar.activation(o1[:], o1[:],
                                             mybir.ActivationFunctionType.Relu)
                        tp = ptp.tile([F_HID, P], F32, tag="tp")
                        nc.tensor.transpose(tp[:], o1[:], ident[:])
                        hT = sb.tile([F_HID, P], BF16, tag="hT")
                        nc.scalar.copy(hT[:], tp[:])
                        pn = pn2.tile([P, F_OUT + 2], F32, tag="pn2")
                        nc.tensor.matmul(pn[:], lhsT=hT[:], rhs=w2_t[:],
                                         start=True, stop=True)
                        rs = sb.tile([P, REC], BF16, tag="rs2")
                        nc.vector.tensor_copy(rs[:], stpl_t[:])
                        nc.vector.tensor_copy(rs[:, 1:F_OUT + 2],
                                              pn[:, :F_OUT + 1])
                        nc.sync.dma_start(S2own[t * P:(t + 1) * P, :], rs[:])
                        rd = sb.tile([P, REC], BF16, tag="rd2")
                        nc.vector.tensor_copy(rd[:], dtpl_t[:])
                        nc.vector.tensor_copy(rd[:, 0:1],
                                              pn[:, F_OUT + 1:F_OUT + 2])
                        nc.sync.dma_start(D2[t * P:(t + 1) * P, :], rd[:])
                    else:
                        nc.vector.tensor_tensor(out=o1[:], in0=o1[:],
                                                in1=b2_t[:],
                                                op=mybir.AluOpType.add)
                        ohp_t = ohp.tile([P, G_SLOTS], F32, tag="ohp")
                        nc.vector.tensor_scalar(
                            ohp_t[:], iota_t[:], gid_t[:, t:t + 1], None,
                            mybir.AluOpType.is_equal)
                        nc.tensor.matmul(pool_ps[:], lhsT=ohp_t[:], rhs=o1[:],
                                         start=(ai == 0),
                                         stop=(ai == len(active) - 1))
                if layer == 2:
                    pm = sb.tile([G_SLOTS, F_OUT], F32, tag="pm")
                    nc.vector.tensor_scalar_mul(pm[:], pool_ps[:], rc_t[:, :1])
                    tp2 = ptp.tile([F_OUT, G_SLOTS], F32, tag="tp2")
                    nc.tensor.transpose(tp2[:], pm[:], ident[:])
                    pmT = sb.tile([F_OUT, G_SLOTS], BF16, tag="pmT")
                    nc.scalar.copy(pmT[:], tp2[:])
                    po = pn2.tile([N_CLS, G_SLOTS], F32, tag="po")
                    nc.tensor.matmul(po[:], lhsT=wl_t[:], rhs=pmT[:],
                                     start=True, stop=True)
                    ot = sb.tile([N_CLS, G_SLOTS], F32, tag="ot")
                    nc.vector.tensor_scalar_add(ot[:], po[:], bl_t[:, :1])
                    nc.sync.dma_start(out[:, :], ot[:])

            edge_phase(S1full, D1, 1)
            nc.gpsimd.collective_compute(
                "AllGather", mybir.AluOpType.bypass,
                replica_groups=[list(range(N_CORES))],
                ins=[S2own[:]], outs=[S2full[:]])
            edge_phase(S2full, D2, 2)
    nc.compile()
    return nc


# ------------------------------------------------------------------- helpers
def _shard(batch):
    cnt = np.bincount(batch, minlength=N_GRAPHS)
    csum = np.concatenate([[0], np.cumsum(cnt)])
    targets = np.linspace(0, N, N_CORES + 1)
    gcut = [0]
    for c in range(1, N_CORES):
        gcut.append(int(np.searchsorted(csum, targets[c])))
    gcut.append(N_GRAPHS)
    gcut = np.array(gcut)
    nbase = csum[gcut]
    return cnt, gcut, nbase


def _wrap16(idx):
    """[n] -> [16, n/16] gpsimd wrap layout."""
    return np.ascontiguousarray(idx.reshape(-1, 16).T)


def kernel(x, edge_index, batch, W1, a_src1, a_dst1, b1,
           W2, a_src2, a_dst2, b2, Wlin, blin):
    x = np.asarray(x, np.float32)
    ei = np.asarray(edge_index, np.int64)
    batch = np.asarray(batch, np.int64)
    W1, a_src1, a_dst1, b1 = (np.asarray(a, np.float32)
                              for a in (W1, a_src1, a_dst1, b1))
    W2, a_src2, a_dst2, b2 = (np.asarray(a, np.float32)
                              for a in (W2, a_src2, a_dst2, b2))
    Wlin, blin = np.asarray(Wlin, np.float32), np.asarray(blin, np.float32)

    loops = np.arange(N, dtype=np.int64)
    src = np.concatenate([ei[0], loops]).astype(np.int64)
    dst = np.concatenate([ei[1], loops]).astype(np.int64)

    gcnt, gcut, nbase = _shard(batch)
    nodes = nbase[1:] - nbase[:-1]
    n_tiles = int(-(-nodes.max() // P))
    R_own = (n_tiles + 1) * P
    SENT = n_tiles * P                     # D-table sentinel row (local)

    core_of_node = np.searchsorted(nbase[1:], np.arange(N), side="right")
    pidx = core_of_node * R_own + (np.arange(N) - nbase[core_of_node])

    ecore = core_of_node[dst]
    dloc = dst - nbase[ecore]
    etile = dloc // P
    spidx = pidx[src]
    egrp = (spidx >= HALF).astype(np.int64)

    # per (core, tile, grp) counts -> uniform column structure
    cnt_ctg = np.zeros((N_CORES, n_tiles, 2), np.int64)
    np.add.at(cnt_ctg, (ecore, etile, egrp), 1)
    nb_g = -(-cnt_ctg.max(axis=0) // P)     # [n_tiles, 2]
    nb_lo, nb_hi = nb_g[:, 0], nb_g[:, 1]
    nbt = nb_lo + nb_hi
    cols_pre = np.concatenate([[0], np.cumsum(nbt)]).astype(np.int64)
    TB = int(cols_pre[-1])

    # slot of every edge: stream position = (col_global*128 + part)
    order = np.lexsort((egrp, etile, ecore))
    s_spidx, s_dloc, s_core = spidx[order], dloc[order], ecore[order]
    s_tile, s_grp = etile[order], egrp[order]
    key = (s_core * n_tiles + s_tile) * 2 + s_grp
    start = np.searchsorted(key, np.arange(N_CORES * n_tiles * 2), side="left")
    rank = np.arange(len(key)) - start[key]
    col_in_grp = rank // P
    part = rank % P
    col = cols_pre[s_tile] + np.where(s_grp == 1, nb_lo[s_tile], 0) + col_in_grp
    spos = col * P + part

    idxS = np.zeros((N_CORES, TB * P), np.int64)
    idxD = np.full((N_CORES, TB * P), SENT, np.int64)
    R_full = N_CORES * R_own
    T1_BASE = R_full - HALF
    sval = np.where(s_grp == 1, s_spidx - T1_BASE, s_spidx)
    idxS[s_core, spos] = sval
    idxD[s_core, spos] = s_dloc
    assert idxS.min() >= 0 and idxS.max() < HALF
    assert idxD.max() <= SENT

    sig = (R_own, tuple(nb_lo.tolist()), tuple(nb_hi.tolist()))
    if sig not in _cache:
        _cache[sig] = build(R_own, nb_lo, nb_hi)
    nc = _cache[sig]

    # ---------------- per-core inputs
    w1aug = np.concatenate([W1, (W1 @ a_src1)[:, None],
                            (W1 @ a_dst1)[:, None]], axis=1).astype(np.float32)
    w2aug = np.concatenate([W2, (W2 @ a_src2)[:, None],
                            (W2 @ a_dst2)[:, None]], axis=1).astype(np.float32)
    b1rep = np.broadcast_to(b1, (P, F_HID)).astype(np.float32).copy()
    b2rep = np.broadcast_to(b2, (P, F_OUT)).astype(np.float32).copy()
    iota = np.broadcast_to(np.arange(P, dtype=np.float32),
                           (P, P)).astype(ml_dtypes.bfloat16)
    stpl = np.zeros((P, REC), ml_dtypes.bfloat16)
    stpl[:, 0] = 1.0
    dtpl = np.zeros((P, REC), ml_dtypes.bfloat16)
    dtpl[:, 1] = np.arange(P, dtype=np.float32).astype(ml_dtypes.bfloat16)
    dsen = np.zeros((P, REC), ml_dtypes.bfloat16)
    dsen[:, 1] = 200.0

    gid = batch.astype(np.int64)
    cores = list(range(N_CORES))
    in_maps = []
    for c in cores:
        xT = np.zeros((P, R_own - P), np.float32)
        xT[:, : nodes[c]] = x[nbase[c]:nbase[c + 1]].T
        gidt = np.full((P, n_tiles), 200.0, np.float32)
        gl = gid[nbase[c]:nbase[c + 1]] - gcut[c]
        nn = np.arange(nodes[c])
        gidt[nn % P, nn // P] = gl
        rc = np.ones((G_SLOTS, 1), np.float32)
        ng = gcut[c + 1] - gcut[c]
        rc[:ng, 0] = 1.0 / np.maximum(gcnt[gcut[c]:gcut[c + 1]], 1.0)
        NXT = R_own - P
        C1 = NXT + (F_HID + 2) + (F_OUT + 2) + N_CLS + 1 + F_HID + F_OUT \
            + n_tiles + 1
        blobf = np.zeros((P, C1), ml_dtypes.bfloat16)
        o = NXT
        blobf[:, 0:NXT] = xT
        blobf[:, o:o + F_HID + 2] = w1aug; o += F_HID + 2
        blobf[0:F_HID, o:o + F_OUT + 2] = w2aug; o += F_OUT + 2
        blobf[0:F_OUT, o:o + N_CLS] = Wlin; o += N_CLS
        blobf[0:N_CLS, o] = blin; o += 1
        blobf[:, o:o + F_HID] = b1rep; o += F_HID
        blobf[:, o:o + F_OUT] = b2rep; o += F_OUT
        blobf[:, o:o + n_tiles] = gidt; o += n_tiles
        blobf[0:G_SLOTS, o] = rc[:, 0]; o += 1
        blobb = np.concatenate([iota, stpl, dtpl, dsen], axis=1)
        blobi = np.concatenate([_wrap16(idxS[c]), _wrap16(idxD[c])],
                               axis=1).astype(np.int16)
        in_maps.append({"blobf": blobf, "blobb": np.ascontiguousarray(blobb),
                        "blobi": np.ascontiguousarray(blobi)})

    LAST_LAUNCH_WALLS.clear()
    res = _run(nc, in_maps, cores)
    out = np.empty((N_GRAPHS, N_CLS), np.float32)
    for c in cores:
        lg = res.results[c]["logits"]
        ng = gcut[c + 1] - gcut[c]
        out[gcut[c]:gcut[c + 1]] = lg[:, :ng].T
    return out
